# revision 12
# baseline (speedup 1.0000x reference)
"""Bidirectional Chamfer loss kernel for Trainium2 (8 NeuronCores).

Problem: yhat [4, 8192, 3] f32, y [4, 8192, 3] f32 ->
    sqrt(0.5 * mean_b(mean_n min_m d2 + mean_m min_n d2)), d2 = clamped sq dist.

Sharding: 8 cores = 4 batches x 2 halves of the yhat rows. Each core computes
its 4096 x 8192 distance tile via TensorE matmul and reduces it on-chip;
the tiny cross-core/cross-partition finish happens on the host.

Distance trick: d2 = |a|^2 + |b|^2 - 2 a.b is produced directly by one matmul
with an augmented contraction dim. To get f32-grade accuracy at bf16 matmul
speed, each f32 input is split into hi+lo bf16 parts (a = ah + al) and all four
cross products are kept; the norms are 3-way bf16 splits. K = 18 rows total.

Engine pipeline per [128, 2048] distance tile:
  TensorE: 4 matmuls -> PSUM f32
  ScalarE: copy PSUM -> SBUF fp16 (the only PSUM consumer)
  VectorE: row-min = fp16 tensor-tensor min tree (2x mode) + short reduce
           col-min = fp16 tensor-tensor min accumulate across row chunks
"""

import numpy as np
import ml_dtypes

B, N, M, D = 4, 8192, 8192, 3
NCORES = 8
NR = N // 2          # yhat rows per core
K = 18               # augmented contraction dim
P = 128              # partitions
NCHUNK = NR // P     # 32 row chunks per core
MGRP = 2048          # columns per PSUM group (4 banks)
NGRP = M // MGRP     # 4 m-groups
MMF = 512            # matmul free dim (one PSUM bank of f32)

BF16 = ml_dtypes.bfloat16

_CACHED_NC = None


def _build_bass():
    import concourse.bass as bass
    import concourse.tile as tile
    from concourse import mybir

    nc = bass.Bass()
    f32 = mybir.dt.float32
    f16 = mybir.dt.float16
    bf16 = mybir.dt.bfloat16
    MIN = mybir.AluOpType.min

    wt_d = nc.dram_tensor("wt", [K, NR], bf16, kind="ExternalInput")
    r_d = nc.dram_tensor("r", [K, M], bf16, kind="ExternalInput")
    id_d = nc.dram_tensor("ident", [P, P], f16, kind="ExternalInput")
    outc_d = nc.dram_tensor("outc", [P, M // P], f32, kind="ExternalOutput")
    outr_d = nc.dram_tensor("outr", [P, NCHUNK], f32, kind="ExternalOutput")

    NCAST = 3  # cast ring depth

    with tile.TileContext(nc) as tc:
        with (
            tc.tile_pool(name="consts", bufs=1) as consts,
            tc.tile_pool(name="tree", bufs=2) as treep,
            tc.tile_pool(name="psum", bufs=6, space="PSUM") as psump,
            tc.tile_pool(name="psumt", bufs=2, space="PSUM") as psumt,
        ):
            wt_s = consts.tile([K, NR], bf16)
            r_s = consts.tile([K, M], bf16)
            nc.sync.dma_start(out=wt_s, in_=wt_d[:, :])
            nc.sync.dma_start(out=r_s, in_=r_d[:, :])

            # Manually managed buffers (no pool recycling -> no multi-proc
            # release waits; every compute instruction carries at most one
            # HW semaphore wait, the TPB EVENTS-struct limit).
            cast_ring = consts.tile([P, NCAST, MGRP], f16)
            colacc = consts.tile([P, M], f16)
            # rowpart[p, i, g] = min over m-group g of d2 row (i, p)
            rowpart = consts.tile([P, NCHUNK, NGRP], f32)
            zero_s = consts.tile([P, 1], f32)
            nc.vector.memset(zero_s, 0.0)
            ident_s = consts.tile([P, P], f16)
            nc.sync.dma_start(out=ident_s, in_=id_d[:, :])
            colfin = consts.tile([P, M // P], f32)

            gen = 0
            for g in range(NGRP):
                for i in range(NCHUNK):
                    cview = cast_ring[:, gen % NCAST, :]
                    # ScalarE pre-touch: first accessor of the (recycled)
                    # ring slot, absorbs the WAR-vs-VectorE wait so the
                    # real copies below each keep a single (PE) wait.
                    nc.scalar.copy(out=cview[:, 0:1], in_=zero_s[:])
                    # One matmul per PSUM bank tile, one ScalarE consumer
                    # each: every Matmult keeps a single sync-wait slot.
                    for s in range(MGRP // MMF):
                        pt = psump.tile([P, MMF], f32, tag="pt")
                        nc.tensor.matmul(
                            pt[:],
                            wt_s[:, i * P:(i + 1) * P],
                            r_s[:, g * MGRP + s * MMF: g * MGRP + (s + 1) * MMF],
                            start=True,
                            stop=True,
                        )
                        nc.scalar.copy(
                            out=cview[:, s * MMF:(s + 1) * MMF], in_=pt[:]
                        )

                    # row-direction: fp16 min tree 2048 -> 256, then reduce
                    t1 = treep.tile([P, MGRP // 2], f16, tag="t1")
                    nc.vector.tensor_tensor(
                        t1[:], cview[:, :MGRP // 2], cview[:, MGRP // 2:], op=MIN
                    )
                    t2 = treep.tile([P, MGRP // 4], f16, tag="t2")
                    nc.vector.tensor_tensor(
                        t2[:], t1[:, :MGRP // 4], t1[:, MGRP // 4:], op=MIN
                    )
                    t3 = treep.tile([P, MGRP // 8], f16, tag="t3")
                    nc.vector.tensor_tensor(
                        t3[:], t2[:, :MGRP // 8], t2[:, MGRP // 8:], op=MIN
                    )
                    nc.vector.tensor_reduce(
                        rowpart[:, i, g:g + 1],
                        t3[:],
                        axis=mybir.AxisListType.X,
                        op=MIN,
                    )
                    # col-direction: accumulate min over row chunks
                    cslice = colacc[:, g * MGRP:(g + 1) * MGRP]
                    if i == 0:
                        nc.vector.tensor_copy(cslice, cview[:])
                    else:
                        nc.vector.tensor_tensor(cslice, cview[:], cslice, op=MIN)
                    gen += 1

                # fold this group's col-min over partitions: PE-transpose
                # each 128-col block, then a short VectorE reduce.
                for t in range(MGRP // P):
                    tb = g * (MGRP // P) + t
                    tp = psumt.tile([P, P], f16, tag="tp")
                    nc.tensor.transpose(
                        tp[:],
                        colacc[:, tb * P:(tb + 1) * P],
                        ident_s[:],
                    )
                    nc.vector.tensor_reduce(
                        colfin[:, tb:tb + 1],
                        tp[:],
                        axis=mybir.AxisListType.X,
                        op=MIN,
                    )

            nc.sync.dma_start(out=outc_d[:, :], in_=colfin[:])
            rowfin = consts.tile([P, NCHUNK], f32)
            nc.vector.tensor_reduce(
                rowfin[:],
                rowpart[:],
                axis=mybir.AxisListType.X,
                op=MIN,
            )
            nc.sync.dma_start(out=outr_d[:, :], in_=rowfin[:])

    _split_multi_waits(nc)
    return nc


def _split_multi_waits(nc):
    """Hoist excess semaphore waits into standalone EventSemaphore ops.

    The TPB EVENTS struct holds exactly one wait per instruction; walrus
    rejects compute instructions scheduled with more. Tile occasionally
    emits 2+ (non-transitively-minimal release/WAW waits), so split them:
    a wait-only EventSemaphore on the same engine right before preserves
    semantics exactly.
    """
    import bass_rust
    from concourse import mybir

    n = 0
    for fn in nc.m.functions:
        for blk in fn.blocks:
            out = []
            for ins in blk.instructions:
                si = getattr(ins, "sync_info", None)
                if (
                    si is not None
                    and len(si.on_wait) > 1
                    and getattr(ins, "engine", None) is not None
                ):
                    waits = list(si.on_wait)
                    for w in waits[:-1]:
                        ev = mybir.InstEventSemaphore(
                            name=f"I-msw-{n}", ins=[], outs=[]
                        )
                        n += 1
                        ev.engine = ins.engine
                        ev.sync_info = bass_rust.SyncInfo(
                            on_wait=[w], on_update=[]
                        )
                        out.append(ev)
                    si.on_wait = [waits[-1]]
                out.append(ins)
            blk.instructions[:] = out


def _get_nc():
    global _CACHED_NC
    if _CACHED_NC is None:
        _CACHED_NC = _build_bass()
    return _CACHED_NC


def _split_hi_lo(x32):
    """f32 array -> (hi, lo) bf16 arrays with hi+lo ~ x to ~2^-16 rel."""
    hi = x32.astype(BF16)
    lo = (x32 - hi.astype(np.float32)).astype(BF16)
    return hi, lo


def _split3(x64):
    """f64 array -> three bf16 arrays summing to x to ~2^-24 rel."""
    h1 = x64.astype(BF16)
    r1 = x64 - h1.astype(np.float64)
    h2 = r1.astype(BF16)
    r2 = r1 - h2.astype(np.float64)
    h3 = r2.astype(BF16)
    return h1, h2, h3


def _build_core_inputs(A, Bm):
    """A: [NR, 3] f32 yhat rows; Bm: [M, 3] f32 y rows -> wt [K,NR], r [K,M] bf16."""
    ah, al = _split_hi_lo(A)                      # [NR,3]
    bh, bl = _split_hi_lo(Bm)                     # [M,3]
    a_rep = ah.astype(np.float64) + al.astype(np.float64)
    b_rep = bh.astype(np.float64) + bl.astype(np.float64)
    sqa = (a_rep * a_rep).sum(-1)                 # [NR] f64, exact norm of repr pts
    sqb = (b_rep * b_rep).sum(-1)
    sa1, sa2, sa3 = _split3(sqa)
    sb1, sb2, sb3 = _split3(sqb)

    ones_n = np.ones(NR, dtype=BF16)
    ones_m = np.ones(M, dtype=BF16)
    n2bh = (-2.0 * bh.astype(np.float32)).astype(BF16)   # exact scaling
    n2bl = (-2.0 * bl.astype(np.float32)).astype(BF16)

    wt = np.empty((K, NR), dtype=BF16)
    r = np.empty((K, M), dtype=BF16)
    wt[0:3] = ah.T
    wt[3:6] = ah.T
    wt[6:9] = al.T
    wt[9:12] = al.T
    wt[12], wt[13], wt[14] = sa1, sa2, sa3
    wt[15] = ones_n
    wt[16] = ones_n
    wt[17] = ones_n

    r[0:3] = n2bh.T
    r[3:6] = n2bl.T
    r[6:9] = n2bh.T
    r[9:12] = n2bl.T
    r[12] = ones_m
    r[13] = ones_m
    r[14] = ones_m
    r[15], r[16], r[17] = sb1, sb2, sb3
    return {
        "wt": np.ascontiguousarray(wt),
        "r": np.ascontiguousarray(r),
        "ident": np.eye(P, dtype=np.float16),
    }


def _run_device(inputs, trace=False):
    from concourse.bass_utils import run_bass_kernel_spmd

    yhat = np.asarray(inputs["yhat"], dtype=np.float32)
    y = np.asarray(inputs["y"], dtype=np.float32)
    in_maps = []
    for c in range(NCORES):
        b, h = divmod(c, 2)
        A = yhat[b, h * NR:(h + 1) * NR]
        in_maps.append(_build_core_inputs(A, y[b]))

    nc = _get_nc()
    res = run_bass_kernel_spmd(
        nc, in_maps, core_ids=list(range(NCORES)), trace=trace
    )
    return res


def _finish_host(results):
    fwd = np.empty(B, dtype=np.float64)
    bwd = np.empty(B, dtype=np.float64)
    for b in range(B):
        rowmins = []
        colfins = []
        for h in range(2):
            rs = results[2 * b + h]
            colfins.append(rs["outc"])                         # [128, M // P]
            rowmins.append(rs["outr"])                         # [128, NCHUNK]
        rowvals = np.concatenate(
            [np.maximum(rm, 0.0).astype(np.float64).ravel() for rm in rowmins]
        )  # all 8192 per-row mins (order irrelevant for the mean)
        fwd[b] = rowvals.mean()
        colmin = np.minimum(colfins[0], colfins[1])            # per-m mins
        bwd[b] = np.maximum(colmin, 0.0).astype(np.float64).mean()
    loss = (fwd + bwd).mean()
    return np.asarray(np.sqrt(0.5 * loss), dtype=np.float32)


def kernel(**inputs):
    res = _run_device(inputs, trace=False)
    return _finish_host(res.results)
